# revision 24
# baseline (speedup 1.0000x reference)
"""Trainium2 Bass kernel for nn_Encoder_6 (conv+GN+InterpLnr x3 -> biLSTM).

Self-contained: host-side prep (sharding, interp gather tables, weight
repacking) + Bass/Tile device kernel + output gather.

Data-parallel over 8 NeuronCores: 64 samples per core.

Device dataflow per core (all samples resident on-chip after one load):
  - activations live in [channel(partition), sample, time] layout
  - conv1d = 10-11 accumulating matmuls per sample-pair (taps x cin-chunks),
    f32r (tf32-like) dtype, PSUM [128, 2x192]
  - GroupNorm stats fused into PSUM evacuation (ACT copy+accum -> sums,
    DVE square+accum -> sumsq), group reduce + expand via tiny matmuls
  - normalize+ReLU = single ACT op (per-partition scale/bias)
  - InterpLnr = gpsimd ap_gather along time + 3 DVE ops (w1*g1 + w2*g2)
  - biLSTM: gate preacts via matmuls straight into PSUM blocks; recurrence
    reads gate slices from PSUM (ACT sigmoid/tanh, DVE cell update)
"""
import sys
from contextlib import ExitStack

sys.path.insert(0, "/opt/trn_rl_repo")

import numpy as np
import ml_dtypes

B = 512
N_CORES = 8
S = B // N_CORES          # samples per core
DIM_PIT = 257
C = 256                   # conv channels
T = 192                   # padded time
TH = 196                  # time with halo (2 each side)
GRP = 16                  # channels per group
DIM_NECK = 32
FREQ = 8
NT_OUT = 24               # output timesteps per direction
MIN_LEN_SEG = 19
MAX_NUM_SEG = 7
W64 = 64                  # 2*MAX_LEN_SEG
EPS = 1e-5
SG = 32                   # samples per stats group (2 groups per core)
NPAIR = 16                # sample pairs per stats group

_cache = {}


# ---------------------------------------------------------------- host prep

def _interp_tables(scales_u, len_seg_raw, n):
    """Gather idx/w1/w2 per sample for one interp layer (numpy, exact)."""
    scales = scales_u.astype(np.float32) + np.float32(0.5)
    j = np.arange(W64, dtype=np.float32)
    idx_scaled = j[None, :] / scales[:, None]
    idx_fl = np.floor(idx_scaled)
    lam = idx_scaled - idx_fl
    len_seg = (len_seg_raw + MIN_LEN_SEG).astype(np.float32)[:, None]
    idx_mask = idx_fl < (len_seg - 1.0)
    ls = (len_seg_raw + MIN_LEN_SEG).reshape(n, MAX_NUM_SEG)
    offset = np.cumsum(ls, axis=-1)
    offset = np.pad(offset[:, :-1], ((0, 0), (1, 0))).reshape(-1, 1)
    idx_org = idx_fl + offset.astype(np.float32)
    mask = (idx_mask & (idx_org < (T - 1))).reshape(n, MAX_NUM_SEG * W64)
    idx_b = np.clip(idx_org.reshape(n, -1).astype(np.int32), 0, T - 2)
    lam_b = lam.reshape(n, -1)
    idx = np.zeros((n, T), np.int32)
    w1 = np.zeros((n, T), np.float32)
    w2 = np.zeros((n, T), np.float32)
    for b in range(n):
        js = np.nonzero(mask[b])[0][:T]
        k = len(js)
        idx[b, :k] = idx_b[b, js]
        w1[b, :k] = 1.0 - lam_b[b, js]
        w2[b, :k] = lam_b[b, js]
    return idx, w1, w2


def _wrap_idx(idx_pairs):
    """[n, NI] int -> ap_gather wrapped layout [n, 128, NI//16] int16."""
    n, NI = idx_pairs.shape
    wrapped = idx_pairs.reshape(n, NI // 16, 16).transpose(0, 2, 1)
    out = np.tile(wrapped[:, None, :, :], (1, 8, 1, 1)).reshape(n, 128, NI // 16)
    return np.ascontiguousarray(out.astype(np.int16))


def _prep_host(inputs):
    """Build per-core input dicts. Returns list of 8 dicts."""
    x = np.asarray(inputs["x"], np.float32)
    scales = np.asarray(inputs["scales"], np.float32)
    lsr = np.asarray(inputs["len_seg_raw"], np.int32)

    # conv weights as lhsT tiles [l, chunk, tap, half, cin128, cout128]
    wconv = np.zeros((3, 2, 5, 2, 128, 128), np.float32)
    for l in range(3):
        w = np.asarray(inputs[f"conv{l}_w"], np.float32)  # [256, cin, 5]
        for cc in range(2):
            for k in range(5):
                for h in range(2):
                    wconv[l, cc, k, h] = w[h * 128:(h + 1) * 128,
                                           cc * 128:(cc + 1) * 128, k].T
    wconv = np.ascontiguousarray(wconv.astype(np.float16))
    # conv0 channel 256 as [5, 256] lhsT (k=tap)
    w0 = np.asarray(inputs["conv0_w"], np.float32)
    wc0e = np.ascontiguousarray(w0[:, 256, :].T.astype(np.float16))  # [5, 256]

    conv_bias = [np.asarray(inputs[f"conv{l}_b"], np.float32) for l in range(3)]
    assert all(np.abs(b).max() == 0.0 for b in conv_bias), \
        "nonzero conv bias not implemented in device kernel"

    gamma_t = np.stack([np.asarray(inputs[f"gn{l}_g"], np.float32).reshape(2, 128)
                        for l in range(3)])          # [3, 2, 128]
    beta_t = np.stack([np.asarray(inputs[f"gn{l}_b"], np.float32).reshape(2, 128)
                       for l in range(3)])
    gamma_t = np.ascontiguousarray(gamma_t.transpose(2, 0, 1).reshape(128, 6))
    beta_t = np.ascontiguousarray(beta_t.transpose(2, 0, 1).reshape(128, 6))

    gind = np.zeros((128, 8), np.float32)
    for c in range(128):
        gind[c, c // 16] = 1.0
    gexp = np.ascontiguousarray(gind.T)               # [8, 128]

    # interp tables, all samples
    idx_all, w1_all, w2_all = [], [], []
    for l in range(3):
        idx, w1, w2 = _interp_tables(scales[l], lsr[l], B)
        idx_all.append(idx)
        w1_all.append(w1)
        w2_all.append(w2)

    # LSTM weights: gate reorder i,f,o,g; gate-partition layouts
    def reord(a):  # [128, ...] gate-major
        i_, f_, g_, o_ = np.split(a, 4, axis=0)
        return np.concatenate([i_, f_, o_, g_], axis=0)

    # gate-partition LSTM layouts (state kept as [gate/h, sample]):
    #  wihg [cin128, d, cc, gate]  lhsT of xw matmuls (fp16)
    #  whhs [h, d, gate]           lhsT of recurrence matmuls (fp16)
    #  lbias [gate, d]             per-partition ACT bias (f32)
    wihg = np.zeros((128, 2, 2, 128), np.float32)
    whhs = np.zeros((32, 2, 128), np.float32)
    lbias = np.zeros((128, 2), np.float32)
    for d, nm in enumerate(["f", "b"]):
        wi = reord(np.asarray(inputs[f"w_ih_{nm}"], np.float32))   # [128, 256]
        wh = reord(np.asarray(inputs[f"w_hh_{nm}"], np.float32))   # [128, 32]
        bb = reord((np.asarray(inputs[f"b_ih_{nm}"], np.float32)
                    + np.asarray(inputs[f"b_hh_{nm}"], np.float32))[:, None])[:, 0]
        for cc in range(2):
            wihg[:, d, cc, :] = wi[:, cc * 128:(cc + 1) * 128].T
        whhs[:, d, :] = wh.T
        lbias[:, d] = bb
    # g-gate rows evaluate tanh(x) as 2*sigmoid(2x)-1: double scale AND bias
    lbias[96:128, :] *= 2.0
    wihg = np.ascontiguousarray(wihg.astype(np.float16))
    whhs = np.ascontiguousarray(whhs.astype(np.float16))
    lbias = np.ascontiguousarray(lbias)

    in_maps = []
    for core in range(N_CORES):
        s0 = core * S
        xs = x[s0:s0 + S]                              # [S, 257, 192]
        xt = xs.transpose(1, 0, 2)                     # [257, S, 192]
        xa = np.zeros((128, S, TH), np.float32)
        xb = np.zeros((128, S, TH), np.float32)
        xa[:, :, 2:194] = xt[:128]
        xb[:, :, 2:194] = xt[128:256]
        xc = np.zeros((5, S, T), np.float32)
        x256 = xt[256]                                 # [S, 192]
        for k in range(5):
            sh = k - 2
            lo, hi = max(0, -sh), min(T, T - sh)
            xc[k, :, lo:hi] = x256[:, lo + sh:hi + sh]

        # banded interp matrices S[t_in, t_out] per (layer, sample), fp16
        wS = np.zeros((3, S, T, T), np.float16)
        bi = np.arange(S)[:, None]
        pj = np.arange(T)[None, :]
        for l in range(3):
            idx = idx_all[l][s0:s0 + S]
            Sm = np.zeros((S, T, T), np.float32)
            Sm[bi, idx, pj] = w1_all[l][s0:s0 + S]
            Sm[bi, idx + 1, pj] += w2_all[l][s0:s0 + S]
            wS[l] = Sm.astype(np.float16)

        in_maps.append({
            "xa": np.ascontiguousarray(xa.astype(np.float16)),
            "xb": np.ascontiguousarray(xb.astype(np.float16)),
            "xc": np.ascontiguousarray(xc.astype(np.float16)),
            "wconv": wconv,
            "wc0e": wc0e,
            "gamma_t": gamma_t,
            "beta_t": beta_t,
            "gind": gind,
            "gexp": gexp,
            "wS": np.ascontiguousarray(wS),
            "id128": np.eye(128, dtype=np.float16),
            "si2": np.ascontiguousarray(
                np.concatenate([np.eye(32), np.eye(32)]).astype(np.float32)),
            "wihg": wihg,
            "whhs": whhs,
            "lbias": lbias,
        })
    return in_maps


# ------------------------------------------------------------- device build

def _build(probe_layer=-1):
    """Build the Bacc module. probe_layer >= 0 adds a probe output of XBUF
    after that layer's interp (for debugging)."""
    import concourse.bass as bass
    import concourse.tile as tile
    from concourse import bacc, mybir

    f32 = mybir.dt.float32
    f32r = mybir.dt.float32r
    bf16 = mybir.dt.bfloat16
    fp16 = mybir.dt.float16
    i16 = mybir.dt.int16
    AF = mybir.ActivationFunctionType
    OP = mybir.AluOpType

    nc = bacc.Bacc("TRN2", target_bir_lowering=False, debug=False,
                   enable_asserts=False, num_devices=N_CORES)

    # DRAM tensors
    d_xa = nc.dram_tensor("xa", [128, S, TH], fp16, kind="ExternalInput")
    d_xb = nc.dram_tensor("xb", [128, S, TH], fp16, kind="ExternalInput")
    d_xc = nc.dram_tensor("xc", [5, S, T], fp16, kind="ExternalInput")
    d_wconv = nc.dram_tensor("wconv", [3, 2, 5, 2, 128, 128], fp16,
                             kind="ExternalInput")
    d_wc0e = nc.dram_tensor("wc0e", [5, 256], fp16, kind="ExternalInput")
    d_gamma = nc.dram_tensor("gamma_t", [128, 6], f32, kind="ExternalInput")
    d_beta = nc.dram_tensor("beta_t", [128, 6], f32, kind="ExternalInput")
    d_gind = nc.dram_tensor("gind", [128, 8], f32, kind="ExternalInput")
    d_gexp = nc.dram_tensor("gexp", [8, 128], f32, kind="ExternalInput")
    d_wS = nc.dram_tensor("wS", [3, S, T, T], fp16, kind="ExternalInput")
    d_id128 = nc.dram_tensor("id128", [128, 128], fp16, kind="ExternalInput")
    d_si2 = nc.dram_tensor("si2", [64, 32], f32, kind="ExternalInput")
    d_wihg = nc.dram_tensor("wihg", [128, 2, 2, 128], fp16, kind="ExternalInput")
    d_whhs = nc.dram_tensor("whhs", [32, 2, 128], fp16, kind="ExternalInput")
    d_lbias = nc.dram_tensor("lbias", [128, 2], f32, kind="ExternalInput")
    d_out = nc.dram_tensor("out", [S, NT_OUT, 64], f32, kind="ExternalOutput")
    d_probe = None
    if probe_layer >= 0:
        d_probe = nc.dram_tensor("probe", [2, 128, S, TH], f32r,
                                 kind="ExternalOutput")

    es = ExitStack()
    with tile.TileContext(nc) as tc, es:
        consts = es.enter_context(tc.tile_pool(name="consts", bufs=1))
        xbufs = es.enter_context(tc.tile_pool(name="xbufs", bufs=1))

        # ---- constants
        t_xc = consts.tile([5, S, T], fp16)
        nc.sync.dma_start(out=t_xc[:], in_=d_xc[:, :, :])
        t_wc0e = consts.tile([5, 256], fp16)
        nc.sync.dma_start(out=t_wc0e[:], in_=d_wc0e[:, :])
        t_gamma = consts.tile([128, 6], f32)
        nc.sync.dma_start(out=t_gamma[:], in_=d_gamma[:, :])
        t_beta = consts.tile([128, 6], f32)
        nc.sync.dma_start(out=t_beta[:], in_=d_beta[:, :])
        t_gind = consts.tile([128, 8], f32)
        nc.sync.dma_start(out=t_gind[:], in_=d_gind[:, :])
        t_gexp = consts.tile([8, 128], f32)
        nc.sync.dma_start(out=t_gexp[:], in_=d_gexp[:, :])
        t_eps = consts.tile([8, 1], f32)
        nc.vector.memset(t_eps[:], EPS)
        t_wihg = consts.tile([128, 2, 2, 128], fp16)
        nc.sync.dma_start(out=t_wihg[:], in_=d_wihg[:, :, :, :])
        t_whhs = consts.tile([32, 2, 128], fp16)
        nc.sync.dma_start(out=t_whhs[:], in_=d_whhs[:, :, :])
        t_lbias = consts.tile([128, 2], f32)
        nc.sync.dma_start(out=t_lbias[:], in_=d_lbias[:, :])
        t_id128 = consts.tile([128, 128], fp16)
        nc.sync.dma_start(out=t_id128[:], in_=d_id128[:, :])
        t_si2 = consts.tile([64, 32], f32)
        nc.sync.dma_start(out=t_si2[:], in_=d_si2[:, :])
        t_gsc = consts.tile([128, 1], f32)
        nc.vector.memset(t_gsc[0:96, :], 1.0)
        nc.vector.memset(t_gsc[96:128, :], 2.0)

        # ---- input activations (xbuf reused as interp output every layer)
        t_xa = xbufs.tile([128, S, TH], fp16)
        t_xb = xbufs.tile([128, S, TH], fp16)
        nc.sync.dma_start(out=t_xa[:], in_=d_xa[:, :, :])
        nc.sync.dma_start(out=t_xb[:], in_=d_xb[:, :, :])
        xbuf = [t_xa, t_xb]

        def mm(out, lhsT, rhs, start, stop, dt=None, **kw):
            if dt is not None:
                lhsT = lhsT.bitcast(dt)
                rhs = rhs.bitcast(dt)
            nc.tensor.matmul(out=out, lhsT=lhsT, rhs=rhs, start=start,
                             stop=stop, **kw)

        # ================= conv + GN + interp layers =================
        with ExitStack() as ces:
            wpool = ces.enter_context(tc.tile_pool(name="wpool", bufs=2))
            hraw_p = ces.enter_context(tc.tile_pool(name="hraw", bufs=2))
            stats_p = ces.enter_context(tc.tile_pool(name="stats", bufs=2))
            small_p = ces.enter_context(tc.tile_pool(name="small", bufs=2))
            y_p = ces.enter_context(tc.tile_pool(name="ybuf", bufs=3))
            sm_p = ces.enter_context(tc.tile_pool(name="smat", bufs=2))
            yt_p = ces.enter_context(tc.tile_pool(name="ytp", bufs=3))
            cpsum = ces.enter_context(
                tc.tile_pool(name="cpsum", bufs=2, space="PSUM"))
            stps = ces.enter_context(
                tc.tile_pool(name="stps", bufs=2, space="PSUM"))
            tpsum = ces.enter_context(
                tc.tile_pool(name="tpsum", bufs=2, space="PSUM"))
            sops = ces.enter_context(
                tc.tile_pool(name="sops", bufs=2, space="PSUM"))

            def emit_ph1_pair(l, grp, pp, t_wc, hraw, st):
                pr = grp * NPAIR + pp
                for h in range(2):
                    ps = cpsum.tile([128, 2, T], f32, tag="cps")
                    ops = []
                    for cc in range(2):
                        for k in range(5):
                            ops.append((
                                t_wc[:, (cc * 5 + k) * 2 + h, :],
                                xbuf[cc][:, 2 * pr:2 * pr + 2, k:k + T]))
                    if l == 0:
                        ops.append((t_wc0e[:, h * 128:(h + 1) * 128],
                                    t_xc[:, 2 * pr:2 * pr + 2, :]))
                    for j, (lh, rh) in enumerate(ops):
                        mm(ps[:], lh, rh, j == 0, j == len(ops) - 1)
                    nc.scalar.activation(
                        out=hraw[h][:, 2 * pp:2 * pp + 2, :],
                        in_=ps[:, :, :], func=AF.Identity)
                    for i in range(2):
                        nc.vector.bn_stats(out=st[h][:, 2 * pp + i, :, :],
                                           in_=ps[:, i, :])

            def emit_ph2(l, st):
                AB = []
                for h in range(2):
                    sq = stats_p.tile([128, SG, 2], f32, tag=f"sq{h}",
                                      name=f"sq{h}")
                    nc.vector.tensor_tensor(out=sq[:], in0=st[h][:, :, :, 1],
                                            in1=st[h][:, :, :, 1], op=OP.mult)
                    g6 = stps.tile([8, SG, 2, 3], f32, tag="gg")
                    mm(g6[:], t_gind[:], st[h][:], True, True)
                    gq = stps.tile([8, SG, 2], f32, tag="gg")
                    mm(gq[:], t_gind[:], sq[:], True, True)
                    g6s = small_p.tile([8, SG, 2, 3], f32, tag="g6s")
                    nc.vector.tensor_copy(out=g6s[:], in_=g6[:])
                    gqs = small_p.tile([8, SG, 2], f32, tag="gqs")
                    nc.vector.tensor_copy(out=gqs[:], in_=gq[:])
                    s_m = small_p.tile([8, SG], f32, tag="s_m")
                    nc.vector.tensor_tensor(out=s_m[:], in0=g6s[:, :, 0, 1],
                                            in1=g6s[:, :, 1, 1], op=OP.add)
                    s_cv = small_p.tile([8, SG], f32, tag="s_cv")
                    nc.vector.tensor_tensor(out=s_cv[:], in0=g6s[:, :, 0, 2],
                                            in1=g6s[:, :, 1, 2], op=OP.add)
                    s_q = small_p.tile([8, SG], f32, tag="s_q")
                    nc.vector.tensor_tensor(out=s_q[:], in0=gqs[:, :, 0],
                                            in1=gqs[:, :, 1], op=OP.add)
                    mean = small_p.tile([8, SG], f32, tag="mean")
                    nc.vector.tensor_scalar_mul(mean[:], s_m[:],
                                                1.0 / (2 * GRP))
                    msq = small_p.tile([8, SG], f32, tag="msq")
                    nc.vector.tensor_tensor(out=msq[:], in0=mean[:],
                                            in1=mean[:], op=OP.mult)
                    t1 = small_p.tile([8, SG], f32, tag="t1")
                    nc.vector.scalar_tensor_tensor(
                        out=t1[:], in0=s_q[:], scalar=1.0 / (2 * GRP),
                        in1=msq[:], op0=OP.mult, op1=OP.subtract)
                    var = small_p.tile([8, SG], f32, tag="var")
                    nc.vector.scalar_tensor_tensor(
                        out=var[:], in0=s_cv[:], scalar=1.0 / (GRP * T),
                        in1=t1[:], op0=OP.mult, op1=OP.add)
                    sd = small_p.tile([8, SG], f32, tag="sd")
                    nc.scalar.activation(out=sd[:], in_=var[:], func=AF.Sqrt,
                                         bias=t_eps[:, :1], scale=1.0)
                    rstd = small_p.tile([8, SG], f32, tag="rstd")
                    nc.vector.reciprocal(rstd[:], sd[:])
                    rp = stps.tile([128, SG], f32, tag="gg")
                    mm(rp[:], t_gexp[:], rstd[:], True, True)
                    mp = stps.tile([128, SG], f32, tag="gg")
                    mm(mp[:], t_gexp[:], mean[:], True, True)
                    At = small_p.tile([128, SG], f32, tag="A")
                    nc.vector.tensor_scalar_mul(
                        At[:], rp[:], t_gamma[:, l * 2 + h:l * 2 + h + 1])
                    tmp = small_p.tile([128, SG], f32, tag="tmp")
                    nc.vector.tensor_tensor(out=tmp[:], in0=mp[:], in1=At[:],
                                            op=OP.mult)
                    Bt = small_p.tile([128, SG], f32, tag="B")
                    nc.vector.tensor_scalar(
                        out=Bt[:], in0=tmp[:], scalar1=-1.0,
                        scalar2=t_beta[:, l * 2 + h:l * 2 + h + 1],
                        op0=OP.mult, op1=OP.add)
                    AB.append((At, Bt))
                return AB

            def emit_ph3_pair(l, grp, pp, hraw, AB):
                pr = grp * NPAIR + pp
                s128 = sm_p.tile([128, 2, T], fp16, tag="s128")
                nc.gpsimd.dma_start(
                    out=s128[:],
                    in_=d_wS[l, 2 * pr:2 * pr + 2, 0:128, :].rearrange(
                        "s t w -> t s w"))
                s64 = sm_p.tile([64, 2, T], fp16, tag="s64")
                nc.gpsimd.dma_start(
                    out=s64[:],
                    in_=d_wS[l, 2 * pr:2 * pr + 2, 128:192, :].rearrange(
                        "s t w -> t s w"))
                for i in range(2):
                    sl = pp * 2 + i
                    sg_ = 2 * pr + i
                    yt128 = yt_p.tile([128, 2, 128], fp16, tag="yt128")
                    yt64 = yt_p.tile([64, 2, 128], fp16, tag="yt64")
                    ptp = tpsum.tile([128, 2, 2, 128], fp16, tag="tp",
                                     name="ptp")
                    for h in range(2):
                        At, Bt = AB[h]
                        yb = y_p.tile([128, T], fp16, tag="yb")
                        nc.scalar.activation(
                            out=yb[:], in_=hraw[h][:, sl, :], func=AF.Relu,
                            bias=Bt[:, sl:sl + 1], scale=At[:, sl:sl + 1])
                        nc.tensor.transpose(out=ptp[:, h, 0, :],
                                            in_=yb[:, 0:128],
                                            identity=t_id128[:])
                        nc.tensor.transpose(out=ptp[0:64, h, 1, :],
                                            in_=yb[:, 128:192],
                                            identity=t_id128[:])
                    nc.scalar.activation(out=yt128[:], in_=ptp[:, :, 0, :],
                                         func=AF.Identity)
                    nc.vector.tensor_copy(out=yt64[:], in_=ptp[0:64, :, 1, :])
                    sout = sops.tile([128, 2, T], f32, tag="so", name="sout")
                    for ch in range(2):
                        mm(sout[:, ch, :], yt128[:, ch, :], s128[:, i, :],
                           True, False)
                        mm(sout[:, ch, :], yt64[:, ch, :], s64[:, i, :],
                           False, True)
                    nc.vector.tensor_copy(out=xbuf[0][:, sg_, 2:194],
                                          in_=sout[:, 0, :])
                    nc.vector.tensor_copy(out=xbuf[1][:, sg_, 2:194],
                                          in_=sout[:, 1, :])

            def emit_probe(l):
                if probe_layer == l:
                    for h in range(2):
                        nc.sync.dma_start(out=d_probe[h, :, :, :],
                                          in_=xbuf[h][:, :, :])

            # software pipeline over the 6 (layer, group) stages: the conv
            # matmuls of stage k+1 are emitted pair-interleaved with the
            # normalize/interp work of stage k so the PE never drains
            # (consecutive stages touch disjoint sample groups).
            prev = None
            t_wc = None
            for l in range(3):
                for grp in range(2):
                    if grp == 0:
                        t_wc = wpool.tile([128, 20, 128], fp16, tag="wconv")
                        nc.sync.dma_start(
                            out=t_wc[:],
                            in_=bass.AP(tensor=d_wconv,
                                        offset=l * 20 * 128 * 128,
                                        ap=[[128, 128], [128 * 128, 20],
                                            [1, 128]]))
                    st = [stats_p.tile([128, SG, 2, 3], f32, tag=f"st{h}",
                                       name=f"st{h}") for h in range(2)]
                    hraw = [hraw_p.tile([128, SG, T], fp16, tag=f"hraw{h}",
                                        name=f"hraw{h}") for h in range(2)]
                    for pp in range(NPAIR):
                        if prev is not None:
                            emit_ph3_pair(prev[0], prev[1], pp, prev[2],
                                          prev[3])
                        emit_ph1_pair(l, grp, pp, t_wc, hraw, st)
                    if prev is not None and prev[1] == 1:
                        emit_probe(prev[0])
                    AB = emit_ph2(l, st)
                    prev = (l, grp, hraw, AB)
            for pp in range(NPAIR):
                emit_ph3_pair(prev[0], prev[1], pp, prev[2], prev[3])
            emit_probe(2)

        # ======================= biLSTM =======================
        # gate-partition layout: gates/state live as [gate-or-h row, sample].
        # Gate rows in PSUM preact banks: i 0:32 | f 32:64 | o 64:96 | g 96:128.
        # Partition re-alignment is done by ACT/DVE output shifts (legal: only
        # DVE *inputs* must share a start partition); P_i/P_f are shifted onto
        # the same rows so c' = P_i + P_f is a plain DVE add.
        # xw preacts batched TB timesteps per PSUM bank (matmul free dim 512).
        TB = 8
        NB8 = T // TB            # 24 banks per direction
        lsb = es.enter_context(tc.tile_pool(name="lstm_sbuf", bufs=1))
        t_SIG = [lsb.tile([128, S], fp16, name=f"sig{d}") for d in range(2)]
        t_TG = [lsb.tile([32, S], fp16, name=f"tg{d}") for d in range(2)]
        t_U = [lsb.tile([32, S], fp16, name=f"uu{d}") for d in range(2)]
        # P2 rows 32:64: [pi | pf] (output-shifts align both with c rows)
        t_P2 = [lsb.tile([64, 2, S], fp16, name=f"pp{d}") for d in range(2)]
        # CC rows 32:64 = c (aligned with sigma_f rows for the P_f product)
        t_CC = [lsb.tile([64, S], fp16, name=f"cc{d}") for d in range(2)]
        t_TC = [lsb.tile([96, S], fp16, name=f"tc{d}") for d in range(2)]
        t_H = [lsb.tile([32, S], fp16, name=f"hh{d}") for d in range(2)]
        t_OUTH = [lsb.tile([32, NT_OUT, S], fp16, name=f"outh{d}")
                  for d in range(2)]
        for d in range(2):
            nc.vector.memset(t_H[d][:], 0.0)
            nc.vector.memset(t_CC[d][32:64, :], 0.0)

        with ExitStack() as les:
            lxw = les.enter_context(
                tc.tile_pool(name="lxw", bufs=2, space="PSUM"))

            def xw_block(b, d):
                """gate preacts for timesteps 8b..8b+7 (natural order)."""
                ps = lxw.tile([128, S, TB], f32, tag=f"xw{d}", name=f"xw{d}")
                for cc in range(2):
                    mm(ps[:], t_wihg[:, d, cc, :],
                       xbuf[cc][:, :, 2 + TB * b:2 + TB * b + TB],
                       cc == 0, False)
                return ps

            cur = [xw_block(0, 0), xw_block(NB8 - 1, 1)]
            nxt = [None, None]

            for g in range(T):
                tt = [g, T - 1 - g]
                bj = [(t // TB, t % TB) for t in tt]
                # bank swap on entry (fwd enters at j==0, bwd at j==7)
                for d in range(2):
                    if g > 0 and bj[d][1] == (0 if d == 0 else TB - 1):
                        cur[d] = nxt[d]
                # two fully-independent dir chains, emitted in lockstep
                for d in range(2):
                    b, j = bj[d]
                    mm(cur[d][:, :, j], t_whhs[:, d, :], t_H[d][:], False,
                       j == (TB - 1 if d == 0 else 0), skip_group_check=True)
                # one sigmoid over all 128 gate rows; g rows get scale 2
                # (tanh(x) = 2*sigmoid(2x) - 1, corrected in the P_i STT)
                for d in range(2):
                    j = bj[d][1]
                    nc.scalar.activation(out=t_SIG[d][:],
                                         in_=cur[d][:, :, j],
                                         func=AF.Sigmoid,
                                         bias=t_lbias[:, d:d + 1],
                                         scale=t_gsc[:, 0:1])
                # P_f on gpsimd (rows 32:64 stay on cores 2-3, no crossing)
                for d in range(2):
                    nc.gpsimd.tensor_tensor(out=t_P2[d][32:64, 1, :],
                                            in0=t_SIG[d][32:64, :],
                                            in1=t_CC[d][32:64, :],
                                            op=OP.mult)
                for d in range(2):
                    nc.vector.tensor_copy(out=t_TG[d][:],
                                          in_=t_SIG[d][96:128, :])
                for d in range(2):
                    nc.vector.tensor_tensor(out=t_U[d][:],
                                            in0=t_SIG[d][0:32, :],
                                            in1=t_TG[d][:], op=OP.mult)
                for d in range(2):
                    nc.vector.scalar_tensor_tensor(
                        out=t_P2[d][32:64, 0, :], in0=t_U[d][:], scalar=2.0,
                        in1=t_SIG[d][0:32, :], op0=OP.mult, op1=OP.subtract)
                for d in range(2):
                    nc.vector.tensor_tensor(out=t_CC[d][32:64, :],
                                            in0=t_P2[d][32:64, 0, :],
                                            in1=t_P2[d][32:64, 1, :],
                                            op=OP.add)
                for d in range(2):
                    nc.scalar.activation(out=t_TC[d][64:96, :],
                                         in_=t_CC[d][32:64, :],
                                         func=AF.Tanh)
                for d in range(2):
                    nc.vector.tensor_tensor(out=t_H[d][:],
                                            in0=t_SIG[d][64:96, :],
                                            in1=t_TC[d][64:96, :], op=OP.mult)
                for d in range(2):
                    t = tt[d]
                    if (d == 0 and t % FREQ == FREQ - 1) or \
                       (d == 1 and t % FREQ == 0):
                        nc.gpsimd.tensor_copy(out=t_OUTH[d][:, t // FREQ, :],
                                              in_=t_H[d][:])
                # prefetch next banks early in each bank's life
                if bj[0][1] == 1 and bj[0][0] + 1 < NB8:
                    nxt[0] = xw_block(bj[0][0] + 1, 0)
                if bj[1][1] == TB - 2 and bj[1][0] - 1 >= 0:
                    nxt[1] = xw_block(bj[1][0] - 1, 1)

        # ---- emit outputs: transpose [32 h, (t-pair, s)] -> [(tp, s), h]
        t_OUTF = [lsb.tile([128, NT_OUT // 2, 32], f32, name=f"outf{d}")
                  for d in range(2)]
        with tc.tile_pool(name="lft", bufs=2, space="PSUM") as lft:
            for d in range(2):
                pf = lft.tile([128, NT_OUT // 2, 32], fp16, tag="pf",
                              name="pf")
                for i in range(NT_OUT // 2):
                    nc.tensor.transpose(out=pf[:, i, :],
                                        in_=t_OUTH[d][:, 2 * i:2 * i + 2, :],
                                        identity=t_id128[0:32, 0:32])
                nc.vector.tensor_copy(out=t_OUTF[d][:], in_=pf[:])

        for d in range(2):
            for tp in range(2):
                nc.sync.dma_start(
                    out=bass.AP(tensor=d_out,
                                offset=tp * 64 + d * 32,
                                ap=[[NT_OUT * 64, S], [2 * 64, NT_OUT // 2],
                                    [1, 32]]),
                    in_=t_OUTF[d][tp * 64:tp * 64 + 64, :, :])

    nc.compile()
    return nc


def _get_nc(probe_layer=-1):
    key = ("nc", probe_layer)
    if key not in _cache:
        _cache[key] = _build(probe_layer)
    return _cache[key]


def run_on_cores(inputs, probe_layer=-1, trace=False):
    """Build (cached), run on 8 cores; returns (results, BassKernelResults)."""
    from concourse.bass_utils import run_bass_kernel_spmd

    nc = _get_nc(probe_layer)
    in_maps = _prep_host(inputs)
    last_exc = None
    for _ in range(3):
        try:
            res = run_bass_kernel_spmd(nc, in_maps,
                                       core_ids=list(range(N_CORES)),
                                       trace=trace)
            return res
        except Exception as e:  # transient NRT errors happen; retry
            last_exc = e
    raise last_exc


def assemble_output(res):
    out = np.zeros((B, NT_OUT, 64), np.float32)
    for core in range(N_CORES):
        s0 = core * S
        out[s0:s0 + S] = res.results[core]["out"]
    return out


def kernel(**inputs):
    res = run_on_cores(inputs)
    return assemble_output(res)



# revision 26
# speedup vs baseline: 1.0983x; 1.0983x over previous
"""Trainium2 Bass kernel for nn_Encoder_6 (conv+GN+InterpLnr x3 -> biLSTM).

Self-contained: host-side prep (sharding, interp gather tables, weight
repacking) + Bass/Tile device kernel + output gather.

Data-parallel over 8 NeuronCores: 64 samples per core.

Device dataflow per core (all samples resident on-chip after one load):
  - activations live in [channel(partition), sample, time] layout
  - conv1d = 10-11 accumulating fp16 matmuls per sample-pair (taps x
    cin-chunks), PSUM [128, 2x192]
  - GroupNorm stats from per-sample DVE bn_stats on the conv PSUM; group
    reduce/expand via tiny matmuls; normalize+ReLU = one ACT op per
    (sample, half) with per-partition scale/bias
  - InterpLnr = per-sample banded time-warp matrix matmul (PE transposes
    h, then [t_in, t_out] fp16 matmuls)
  - the six (layer, group) stages are software-pipelined: stage k+1's conv
    matmuls emit pair-interleaved with stage k's normalize/interp so the
    PE never drains (consecutive stages touch disjoint sample groups)
  - biLSTM in gate-partition layout [gate/h row, sample]: xw preacts
    batched 8 steps per PSUM bank (free-dim-512 matmuls), recurrence
    matmul consumes h [h, s] directly (no transpose); partition
    re-alignment via ACT/DVE output shifts; two independent dir chains
"""
import sys
from contextlib import ExitStack

sys.path.insert(0, "/opt/trn_rl_repo")

import numpy as np
import ml_dtypes

B = 512
N_CORES = 8
S = B // N_CORES          # samples per core
DIM_PIT = 257
C = 256                   # conv channels
T = 192                   # padded time
TH = 196                  # time with halo (2 each side)
GRP = 16                  # channels per group
DIM_NECK = 32
FREQ = 8
NT_OUT = 24               # output timesteps per direction
MIN_LEN_SEG = 19
MAX_NUM_SEG = 7
W64 = 64                  # 2*MAX_LEN_SEG
EPS = 1e-5
SG = 32                   # samples per stats group (2 groups per core)
NPAIR = 16                # sample pairs per stats group

_cache = {}


# ---------------------------------------------------------------- host prep

def _interp_tables(scales_u, len_seg_raw, n):
    """Gather idx/w1/w2 per sample for one interp layer (numpy, exact)."""
    scales = scales_u.astype(np.float32) + np.float32(0.5)
    j = np.arange(W64, dtype=np.float32)
    idx_scaled = j[None, :] / scales[:, None]
    idx_fl = np.floor(idx_scaled)
    lam = idx_scaled - idx_fl
    len_seg = (len_seg_raw + MIN_LEN_SEG).astype(np.float32)[:, None]
    idx_mask = idx_fl < (len_seg - 1.0)
    ls = (len_seg_raw + MIN_LEN_SEG).reshape(n, MAX_NUM_SEG)
    offset = np.cumsum(ls, axis=-1)
    offset = np.pad(offset[:, :-1], ((0, 0), (1, 0))).reshape(-1, 1)
    idx_org = idx_fl + offset.astype(np.float32)
    mask = (idx_mask & (idx_org < (T - 1))).reshape(n, MAX_NUM_SEG * W64)
    idx_b = np.clip(idx_org.reshape(n, -1).astype(np.int32), 0, T - 2)
    lam_b = lam.reshape(n, -1)
    idx = np.zeros((n, T), np.int32)
    w1 = np.zeros((n, T), np.float32)
    w2 = np.zeros((n, T), np.float32)
    for b in range(n):
        js = np.nonzero(mask[b])[0][:T]
        k = len(js)
        idx[b, :k] = idx_b[b, js]
        w1[b, :k] = 1.0 - lam_b[b, js]
        w2[b, :k] = lam_b[b, js]
    return idx, w1, w2


def _wrap_idx(idx_pairs):
    """[n, NI] int -> ap_gather wrapped layout [n, 128, NI//16] int16."""
    n, NI = idx_pairs.shape
    wrapped = idx_pairs.reshape(n, NI // 16, 16).transpose(0, 2, 1)
    out = np.tile(wrapped[:, None, :, :], (1, 8, 1, 1)).reshape(n, 128, NI // 16)
    return np.ascontiguousarray(out.astype(np.int16))


def _prep_host(inputs):
    """Build per-core input dicts. Returns list of 8 dicts."""
    x = np.asarray(inputs["x"], np.float32)
    scales = np.asarray(inputs["scales"], np.float32)
    lsr = np.asarray(inputs["len_seg_raw"], np.int32)

    # conv weights as lhsT tiles [l, chunk, tap, half, cin128, cout128]
    wconv = np.zeros((3, 2, 5, 2, 128, 128), np.float32)
    for l in range(3):
        w = np.asarray(inputs[f"conv{l}_w"], np.float32)  # [256, cin, 5]
        for cc in range(2):
            for k in range(5):
                for h in range(2):
                    wconv[l, cc, k, h] = w[h * 128:(h + 1) * 128,
                                           cc * 128:(cc + 1) * 128, k].T
    wconv = np.ascontiguousarray(wconv.astype(np.float16))
    # conv0 channel 256 as [5, 256] lhsT (k=tap)
    w0 = np.asarray(inputs["conv0_w"], np.float32)
    wc0e = np.ascontiguousarray(w0[:, 256, :].T.astype(np.float16))  # [5, 256]

    conv_bias = [np.asarray(inputs[f"conv{l}_b"], np.float32) for l in range(3)]
    assert all(np.abs(b).max() == 0.0 for b in conv_bias), \
        "nonzero conv bias not implemented in device kernel"

    gamma_t = np.stack([np.asarray(inputs[f"gn{l}_g"], np.float32).reshape(2, 128)
                        for l in range(3)])          # [3, 2, 128]
    beta_t = np.stack([np.asarray(inputs[f"gn{l}_b"], np.float32).reshape(2, 128)
                       for l in range(3)])
    gamma_t = np.ascontiguousarray(gamma_t.transpose(2, 0, 1).reshape(128, 6))
    beta_t = np.ascontiguousarray(beta_t.transpose(2, 0, 1).reshape(128, 6))

    gind = np.zeros((128, 8), np.float32)
    for c in range(128):
        gind[c, c // 16] = 1.0
    gexp = np.ascontiguousarray(gind.T)               # [8, 128]

    # interp tables, all samples
    idx_all, w1_all, w2_all = [], [], []
    for l in range(3):
        idx, w1, w2 = _interp_tables(scales[l], lsr[l], B)
        idx_all.append(idx)
        w1_all.append(w1)
        w2_all.append(w2)

    # LSTM weights: gate reorder i,f,o,g; gate-partition layouts
    def reord(a):  # [128, ...] gate-major
        i_, f_, g_, o_ = np.split(a, 4, axis=0)
        return np.concatenate([i_, f_, o_, g_], axis=0)

    # gate-partition LSTM layouts (state kept as [gate/h, sample]):
    #  wihg [cin128, d, cc, gate]  lhsT of xw matmuls (fp16)
    #  whhs [h, d, gate]           lhsT of recurrence matmuls (fp16)
    #  lbias [gate, d]             per-partition ACT bias (f32)
    wihg = np.zeros((128, 2, 2, 128), np.float32)
    whhs = np.zeros((32, 2, 128), np.float32)
    lbias = np.zeros((128, 2), np.float32)
    for d, nm in enumerate(["f", "b"]):
        wi = reord(np.asarray(inputs[f"w_ih_{nm}"], np.float32))   # [128, 256]
        wh = reord(np.asarray(inputs[f"w_hh_{nm}"], np.float32))   # [128, 32]
        bb = reord((np.asarray(inputs[f"b_ih_{nm}"], np.float32)
                    + np.asarray(inputs[f"b_hh_{nm}"], np.float32))[:, None])[:, 0]
        for cc in range(2):
            wihg[:, d, cc, :] = wi[:, cc * 128:(cc + 1) * 128].T
        whhs[:, d, :] = wh.T
        lbias[:, d] = bb
    wihg = np.ascontiguousarray(wihg.astype(np.float16))
    whhs = np.ascontiguousarray(whhs.astype(np.float16))
    lbias = np.ascontiguousarray(lbias)

    in_maps = []
    for core in range(N_CORES):
        s0 = core * S
        xs = x[s0:s0 + S]                              # [S, 257, 192]
        xt = xs.transpose(1, 0, 2)                     # [257, S, 192]
        xa = np.zeros((128, S, TH), np.float32)
        xb = np.zeros((128, S, TH), np.float32)
        xa[:, :, 2:194] = xt[:128]
        xb[:, :, 2:194] = xt[128:256]
        xc = np.zeros((5, S, T), np.float32)
        x256 = xt[256]                                 # [S, 192]
        for k in range(5):
            sh = k - 2
            lo, hi = max(0, -sh), min(T, T - sh)
            xc[k, :, lo:hi] = x256[:, lo + sh:hi + sh]

        # banded interp matrices S[t_in, t_out] per (layer, sample), fp16
        wS = np.zeros((3, S, T, T), np.float16)
        bi = np.arange(S)[:, None]
        pj = np.arange(T)[None, :]
        for l in range(3):
            idx = idx_all[l][s0:s0 + S]
            Sm = np.zeros((S, T, T), np.float32)
            Sm[bi, idx, pj] = w1_all[l][s0:s0 + S]
            Sm[bi, idx + 1, pj] += w2_all[l][s0:s0 + S]
            wS[l] = Sm.astype(np.float16)

        in_maps.append({
            "xa": np.ascontiguousarray(xa.astype(np.float16)),
            "xb": np.ascontiguousarray(xb.astype(np.float16)),
            "xc": np.ascontiguousarray(xc.astype(np.float16)),
            "wconv": wconv,
            "wc0e": wc0e,
            "gamma_t": gamma_t,
            "beta_t": beta_t,
            "gind": gind,
            "gexp": gexp,
            "wS": np.ascontiguousarray(wS),
            "id128": np.eye(128, dtype=np.float16),
            "si2": np.ascontiguousarray(
                np.concatenate([np.eye(32), np.eye(32)]).astype(np.float32)),
            "wihg": wihg,
            "whhs": whhs,
            "lbias": lbias,
        })
    return in_maps


# ------------------------------------------------------------- device build

def _build(probe_layer=-1):
    """Build the Bacc module. probe_layer >= 0 adds a probe output of XBUF
    after that layer's interp (for debugging)."""
    import concourse.bass as bass
    import concourse.tile as tile
    from concourse import bacc, mybir

    f32 = mybir.dt.float32
    f32r = mybir.dt.float32r
    bf16 = mybir.dt.bfloat16
    fp16 = mybir.dt.float16
    i16 = mybir.dt.int16
    AF = mybir.ActivationFunctionType
    OP = mybir.AluOpType

    nc = bacc.Bacc("TRN2", target_bir_lowering=False, debug=False,
                   enable_asserts=False, num_devices=N_CORES)

    # DRAM tensors
    d_xa = nc.dram_tensor("xa", [128, S, TH], fp16, kind="ExternalInput")
    d_xb = nc.dram_tensor("xb", [128, S, TH], fp16, kind="ExternalInput")
    d_xc = nc.dram_tensor("xc", [5, S, T], fp16, kind="ExternalInput")
    d_wconv = nc.dram_tensor("wconv", [3, 2, 5, 2, 128, 128], fp16,
                             kind="ExternalInput")
    d_wc0e = nc.dram_tensor("wc0e", [5, 256], fp16, kind="ExternalInput")
    d_gamma = nc.dram_tensor("gamma_t", [128, 6], f32, kind="ExternalInput")
    d_beta = nc.dram_tensor("beta_t", [128, 6], f32, kind="ExternalInput")
    d_gind = nc.dram_tensor("gind", [128, 8], f32, kind="ExternalInput")
    d_gexp = nc.dram_tensor("gexp", [8, 128], f32, kind="ExternalInput")
    d_wS = nc.dram_tensor("wS", [3, S, T, T], fp16, kind="ExternalInput")
    d_id128 = nc.dram_tensor("id128", [128, 128], fp16, kind="ExternalInput")
    d_si2 = nc.dram_tensor("si2", [64, 32], f32, kind="ExternalInput")
    d_wihg = nc.dram_tensor("wihg", [128, 2, 2, 128], fp16, kind="ExternalInput")
    d_whhs = nc.dram_tensor("whhs", [32, 2, 128], fp16, kind="ExternalInput")
    d_lbias = nc.dram_tensor("lbias", [128, 2], f32, kind="ExternalInput")
    d_out = nc.dram_tensor("out", [S, NT_OUT, 64], f32, kind="ExternalOutput")
    d_probe = None
    if probe_layer >= 0:
        d_probe = nc.dram_tensor("probe", [2, 128, S, TH], f32r,
                                 kind="ExternalOutput")

    es = ExitStack()
    with tile.TileContext(nc) as tc, es:
        consts = es.enter_context(tc.tile_pool(name="consts", bufs=1))
        xbufs = es.enter_context(tc.tile_pool(name="xbufs", bufs=1))

        # ---- constants
        t_xc = consts.tile([5, S, T], fp16)
        nc.sync.dma_start(out=t_xc[:], in_=d_xc[:, :, :])
        t_wc0e = consts.tile([5, 256], fp16)
        nc.sync.dma_start(out=t_wc0e[:], in_=d_wc0e[:, :])
        t_gamma = consts.tile([128, 6], f32)
        nc.sync.dma_start(out=t_gamma[:], in_=d_gamma[:, :])
        t_beta = consts.tile([128, 6], f32)
        nc.sync.dma_start(out=t_beta[:], in_=d_beta[:, :])
        t_gind = consts.tile([128, 8], f32)
        nc.sync.dma_start(out=t_gind[:], in_=d_gind[:, :])
        t_gexp = consts.tile([8, 128], f32)
        nc.sync.dma_start(out=t_gexp[:], in_=d_gexp[:, :])
        t_eps = consts.tile([8, 1], f32)
        nc.vector.memset(t_eps[:], EPS)
        t_wihg = consts.tile([128, 2, 2, 128], fp16)
        nc.sync.dma_start(out=t_wihg[:], in_=d_wihg[:, :, :, :])
        t_whhs = consts.tile([32, 2, 128], fp16)
        nc.sync.dma_start(out=t_whhs[:], in_=d_whhs[:, :, :])
        t_lbias = consts.tile([128, 2], f32)
        nc.sync.dma_start(out=t_lbias[:], in_=d_lbias[:, :])
        t_id128 = consts.tile([128, 128], fp16)
        nc.sync.dma_start(out=t_id128[:], in_=d_id128[:, :])
        t_si2 = consts.tile([64, 32], f32)
        nc.sync.dma_start(out=t_si2[:], in_=d_si2[:, :])

        # ---- input activations (xbuf reused as interp output every layer)
        t_xa = xbufs.tile([128, S, TH], fp16)
        t_xb = xbufs.tile([128, S, TH], fp16)
        nc.sync.dma_start(out=t_xa[:], in_=d_xa[:, :, :])
        nc.sync.dma_start(out=t_xb[:], in_=d_xb[:, :, :])
        xbuf = [t_xa, t_xb]

        def mm(out, lhsT, rhs, start, stop, dt=None, **kw):
            if dt is not None:
                lhsT = lhsT.bitcast(dt)
                rhs = rhs.bitcast(dt)
            nc.tensor.matmul(out=out, lhsT=lhsT, rhs=rhs, start=start,
                             stop=stop, **kw)

        # ================= conv + GN + interp layers =================
        with ExitStack() as ces:
            wpool = ces.enter_context(tc.tile_pool(name="wpool", bufs=2))
            hraw_p = ces.enter_context(tc.tile_pool(name="hraw", bufs=2))
            stats_p = ces.enter_context(tc.tile_pool(name="stats", bufs=2))
            small_p = ces.enter_context(tc.tile_pool(name="small", bufs=2))
            y_p = ces.enter_context(tc.tile_pool(name="ybuf", bufs=3))
            sm_p = ces.enter_context(tc.tile_pool(name="smat", bufs=2))
            yt_p = ces.enter_context(tc.tile_pool(name="ytp", bufs=3))
            cpsum = ces.enter_context(
                tc.tile_pool(name="cpsum", bufs=2, space="PSUM"))
            stps = ces.enter_context(
                tc.tile_pool(name="stps", bufs=2, space="PSUM"))
            tpsum = ces.enter_context(
                tc.tile_pool(name="tpsum", bufs=2, space="PSUM"))
            sops = ces.enter_context(
                tc.tile_pool(name="sops", bufs=2, space="PSUM"))

            def emit_ph1_pair(l, grp, pp, t_wc, hraw, st):
                pr = grp * NPAIR + pp
                for h in range(2):
                    ps = cpsum.tile([128, 2, T], f32, tag="cps")
                    ops = []
                    for cc in range(2):
                        for k in range(5):
                            ops.append((
                                t_wc[:, (cc * 5 + k) * 2 + h, :],
                                xbuf[cc][:, 2 * pr:2 * pr + 2, k:k + T]))
                    if l == 0:
                        ops.append((t_wc0e[:, h * 128:(h + 1) * 128],
                                    t_xc[:, 2 * pr:2 * pr + 2, :]))
                    for j, (lh, rh) in enumerate(ops):
                        mm(ps[:], lh, rh, j == 0, j == len(ops) - 1)
                    nc.scalar.activation(
                        out=hraw[h][:, 2 * pp:2 * pp + 2, :],
                        in_=ps[:, :, :], func=AF.Identity)
                    for i in range(2):
                        nc.vector.bn_stats(out=st[h][:, 2 * pp + i, :, :],
                                           in_=ps[:, i, :])

            def emit_ph2(l, st):
                AB = []
                for h in range(2):
                    sq = stats_p.tile([128, SG, 2], f32, tag=f"sq{h}",
                                      name=f"sq{h}")
                    nc.vector.tensor_tensor(out=sq[:], in0=st[h][:, :, :, 1],
                                            in1=st[h][:, :, :, 1], op=OP.mult)
                    g6 = stps.tile([8, SG, 2, 3], f32, tag="gg")
                    mm(g6[:], t_gind[:], st[h][:], True, True)
                    gq = stps.tile([8, SG, 2], f32, tag="gg")
                    mm(gq[:], t_gind[:], sq[:], True, True)
                    g6s = small_p.tile([8, SG, 2, 3], f32, tag="g6s")
                    nc.vector.tensor_copy(out=g6s[:], in_=g6[:])
                    gqs = small_p.tile([8, SG, 2], f32, tag="gqs")
                    nc.vector.tensor_copy(out=gqs[:], in_=gq[:])
                    s_m = small_p.tile([8, SG], f32, tag="s_m")
                    nc.vector.tensor_tensor(out=s_m[:], in0=g6s[:, :, 0, 1],
                                            in1=g6s[:, :, 1, 1], op=OP.add)
                    s_cv = small_p.tile([8, SG], f32, tag="s_cv")
                    nc.vector.tensor_tensor(out=s_cv[:], in0=g6s[:, :, 0, 2],
                                            in1=g6s[:, :, 1, 2], op=OP.add)
                    s_q = small_p.tile([8, SG], f32, tag="s_q")
                    nc.vector.tensor_tensor(out=s_q[:], in0=gqs[:, :, 0],
                                            in1=gqs[:, :, 1], op=OP.add)
                    mean = small_p.tile([8, SG], f32, tag="mean")
                    nc.vector.tensor_scalar_mul(mean[:], s_m[:],
                                                1.0 / (2 * GRP))
                    msq = small_p.tile([8, SG], f32, tag="msq")
                    nc.vector.tensor_tensor(out=msq[:], in0=mean[:],
                                            in1=mean[:], op=OP.mult)
                    t1 = small_p.tile([8, SG], f32, tag="t1")
                    nc.vector.scalar_tensor_tensor(
                        out=t1[:], in0=s_q[:], scalar=1.0 / (2 * GRP),
                        in1=msq[:], op0=OP.mult, op1=OP.subtract)
                    var = small_p.tile([8, SG], f32, tag="var")
                    nc.vector.scalar_tensor_tensor(
                        out=var[:], in0=s_cv[:], scalar=1.0 / (GRP * T),
                        in1=t1[:], op0=OP.mult, op1=OP.add)
                    sd = small_p.tile([8, SG], f32, tag="sd")
                    nc.scalar.activation(out=sd[:], in_=var[:], func=AF.Sqrt,
                                         bias=t_eps[:, :1], scale=1.0)
                    rstd = small_p.tile([8, SG], f32, tag="rstd")
                    nc.vector.reciprocal(rstd[:], sd[:])
                    rp = stps.tile([128, SG], f32, tag="gg")
                    mm(rp[:], t_gexp[:], rstd[:], True, True)
                    mp = stps.tile([128, SG], f32, tag="gg")
                    mm(mp[:], t_gexp[:], mean[:], True, True)
                    At = small_p.tile([128, SG], f32, tag="A")
                    nc.vector.tensor_scalar_mul(
                        At[:], rp[:], t_gamma[:, l * 2 + h:l * 2 + h + 1])
                    tmp = small_p.tile([128, SG], f32, tag="tmp")
                    nc.vector.tensor_tensor(out=tmp[:], in0=mp[:], in1=At[:],
                                            op=OP.mult)
                    Bt = small_p.tile([128, SG], f32, tag="B")
                    nc.vector.tensor_scalar(
                        out=Bt[:], in0=tmp[:], scalar1=-1.0,
                        scalar2=t_beta[:, l * 2 + h:l * 2 + h + 1],
                        op0=OP.mult, op1=OP.add)
                    AB.append((At, Bt))
                return AB

            def emit_ph3_pair(l, grp, pp, hraw, AB):
                pr = grp * NPAIR + pp
                s128 = sm_p.tile([128, 2, T], fp16, tag="s128")
                nc.gpsimd.dma_start(
                    out=s128[:],
                    in_=d_wS[l, 2 * pr:2 * pr + 2, 0:128, :].rearrange(
                        "s t w -> t s w"))
                s64 = sm_p.tile([64, 2, T], fp16, tag="s64")
                nc.gpsimd.dma_start(
                    out=s64[:],
                    in_=d_wS[l, 2 * pr:2 * pr + 2, 128:192, :].rearrange(
                        "s t w -> t s w"))
                for i in range(2):
                    sl = pp * 2 + i
                    sg_ = 2 * pr + i
                    yt128 = yt_p.tile([128, 2, 128], fp16, tag="yt128")
                    yt64 = yt_p.tile([64, 2, 128], fp16, tag="yt64")
                    ptp = tpsum.tile([128, 2, 2, 128], fp16, tag="tp",
                                     name="ptp")
                    for h in range(2):
                        At, Bt = AB[h]
                        yb = y_p.tile([128, T], fp16, tag="yb")
                        nc.scalar.activation(
                            out=yb[:], in_=hraw[h][:, sl, :], func=AF.Relu,
                            bias=Bt[:, sl:sl + 1], scale=At[:, sl:sl + 1])
                        nc.tensor.transpose(out=ptp[:, h, 0, :],
                                            in_=yb[:, 0:128],
                                            identity=t_id128[:])
                        nc.tensor.transpose(out=ptp[0:64, h, 1, :],
                                            in_=yb[:, 128:192],
                                            identity=t_id128[:])
                    nc.scalar.activation(out=yt128[:], in_=ptp[:, :, 0, :],
                                         func=AF.Identity)
                    nc.vector.tensor_copy(out=yt64[:], in_=ptp[0:64, :, 1, :])
                    sout = sops.tile([128, 2, T], f32, tag="so", name="sout")
                    for ch in range(2):
                        mm(sout[:, ch, :], yt128[:, ch, :], s128[:, i, :],
                           True, False)
                        mm(sout[:, ch, :], yt64[:, ch, :], s64[:, i, :],
                           False, True)
                    nc.vector.tensor_copy(out=xbuf[0][:, sg_, 2:194],
                                          in_=sout[:, 0, :])
                    nc.vector.tensor_copy(out=xbuf[1][:, sg_, 2:194],
                                          in_=sout[:, 1, :])

            def emit_probe(l):
                if probe_layer == l:
                    for h in range(2):
                        nc.sync.dma_start(out=d_probe[h, :, :, :],
                                          in_=xbuf[h][:, :, :])

            # software pipeline over the 6 (layer, group) stages: the conv
            # matmuls of stage k+1 are emitted pair-interleaved with the
            # normalize/interp work of stage k so the PE never drains
            # (consecutive stages touch disjoint sample groups).
            prev = None
            t_wc = None
            for l in range(3):
                for grp in range(2):
                    if grp == 0:
                        t_wc = wpool.tile([128, 20, 128], fp16, tag="wconv")
                        nc.sync.dma_start(
                            out=t_wc[:],
                            in_=bass.AP(tensor=d_wconv,
                                        offset=l * 20 * 128 * 128,
                                        ap=[[128, 128], [128 * 128, 20],
                                            [1, 128]]))
                    st = [stats_p.tile([128, SG, 2, 3], f32, tag=f"st{h}",
                                       name=f"st{h}") for h in range(2)]
                    hraw = [hraw_p.tile([128, SG, T], fp16, tag=f"hraw{h}",
                                        name=f"hraw{h}") for h in range(2)]
                    for pp in range(NPAIR):
                        if prev is not None:
                            emit_ph3_pair(prev[0], prev[1], pp, prev[2],
                                          prev[3])
                        emit_ph1_pair(l, grp, pp, t_wc, hraw, st)
                    if prev is not None and prev[1] == 1:
                        emit_probe(prev[0])
                    AB = emit_ph2(l, st)
                    prev = (l, grp, hraw, AB)
            for pp in range(NPAIR):
                emit_ph3_pair(prev[0], prev[1], pp, prev[2], prev[3])
            emit_probe(2)

        # ======================= biLSTM =======================
        # gate-partition layout: gates/state live as [gate-or-h row, sample].
        # Gate rows in PSUM preact banks: i 0:32 | f 32:64 | o 64:96 | g 96:128.
        # Partition re-alignment is done by ACT/DVE output shifts (legal: only
        # DVE *inputs* must share a start partition); P_i/P_f are shifted onto
        # the same rows so c' = P_i + P_f is a plain DVE add.
        # xw preacts batched TB timesteps per PSUM bank (matmul free dim 512).
        TB = 8
        NB8 = T // TB            # 24 banks per direction
        lsb = es.enter_context(tc.tile_pool(name="lstm_sbuf", bufs=1))
        t_SIG = [lsb.tile([96, S], fp16, name=f"sig{d}") for d in range(2)]
        t_TG = [lsb.tile([32, S], fp16, name=f"tg{d}") for d in range(2)]
        # P2: [pi/pf, s] both at rows 0:32 (DVE output-shift realigns P_f)
        t_P2 = [lsb.tile([32, 2, S], fp16, name=f"pp{d}") for d in range(2)]
        # CC rows 32:64 = c (aligned with sigma_f rows for the P_f product)
        t_CC = [lsb.tile([64, S], fp16, name=f"cc{d}") for d in range(2)]
        t_TC = [lsb.tile([96, S], fp16, name=f"tc{d}") for d in range(2)]
        t_H = [lsb.tile([32, S], fp16, name=f"hh{d}") for d in range(2)]
        t_OUTH = [lsb.tile([32, NT_OUT, S], fp16, name=f"outh{d}")
                  for d in range(2)]
        for d in range(2):
            nc.vector.memset(t_H[d][:], 0.0)
            nc.vector.memset(t_CC[d][32:64, :], 0.0)

        with ExitStack() as les:
            lxw = les.enter_context(
                tc.tile_pool(name="lxw", bufs=2, space="PSUM"))

            def xw_block(b, d):
                """gate preacts for timesteps 8b..8b+7 (natural order)."""
                ps = lxw.tile([128, S, TB], f32, tag=f"xw{d}", name=f"xw{d}")
                for cc in range(2):
                    mm(ps[:], t_wihg[:, d, cc, :],
                       xbuf[cc][:, :, 2 + TB * b:2 + TB * b + TB],
                       cc == 0, False)
                return ps

            cur = [xw_block(0, 0), xw_block(NB8 - 1, 1)]
            nxt = [None, None]

            for g in range(T):
                tt = [g, T - 1 - g]
                bj = [(t // TB, t % TB) for t in tt]
                # bank swap on entry (fwd enters at j==0, bwd at j==7)
                for d in range(2):
                    if g > 0 and bj[d][1] == (0 if d == 0 else TB - 1):
                        cur[d] = nxt[d]
                # two fully-independent dir chains, emitted in lockstep
                for d in range(2):
                    b, j = bj[d]
                    mm(cur[d][:, :, j], t_whhs[:, d, :], t_H[d][:], False,
                       j == (TB - 1 if d == 0 else 0), skip_group_check=True)
                for d in range(2):
                    j = bj[d][1]
                    nc.scalar.activation(out=t_SIG[d][:], in_=cur[d][0:96, :, j],
                                         func=AF.Sigmoid,
                                         bias=t_lbias[0:96, d:d + 1])
                for d in range(2):
                    j = bj[d][1]
                    nc.scalar.activation(out=t_TG[d][:],
                                         in_=cur[d][96:128, :, j],
                                         func=AF.Tanh,
                                         bias=t_lbias[96:128, d:d + 1])
                for d in range(2):
                    nc.vector.tensor_tensor(out=t_P2[d][:, 0, :],
                                            in0=t_SIG[d][0:32, :],
                                            in1=t_TG[d][:], op=OP.mult)
                for d in range(2):
                    nc.vector.tensor_tensor(out=t_P2[d][:, 1, :],
                                            in0=t_SIG[d][32:64, :],
                                            in1=t_CC[d][32:64, :],
                                            op=OP.mult)
                for d in range(2):
                    nc.vector.tensor_tensor(out=t_CC[d][32:64, :],
                                            in0=t_P2[d][:, 0, :],
                                            in1=t_P2[d][:, 1, :], op=OP.add)
                for d in range(2):
                    nc.scalar.activation(out=t_TC[d][64:96, :],
                                         in_=t_CC[d][32:64, :],
                                         func=AF.Tanh)
                for d in range(2):
                    nc.vector.tensor_tensor(out=t_H[d][:],
                                            in0=t_SIG[d][64:96, :],
                                            in1=t_TC[d][64:96, :], op=OP.mult)
                for d in range(2):
                    t = tt[d]
                    if (d == 0 and t % FREQ == FREQ - 1) or \
                       (d == 1 and t % FREQ == 0):
                        nc.vector.tensor_copy(out=t_OUTH[d][:, t // FREQ, :],
                                              in_=t_H[d][:])
                # prefetch next banks early in each bank's life
                if bj[0][1] == 1 and bj[0][0] + 1 < NB8:
                    nxt[0] = xw_block(bj[0][0] + 1, 0)
                if bj[1][1] == TB - 2 and bj[1][0] - 1 >= 0:
                    nxt[1] = xw_block(bj[1][0] - 1, 1)

        # ---- emit outputs: transpose [32 h, (t-pair, s)] -> [(tp, s), h]
        t_OUTF = [lsb.tile([128, NT_OUT // 2, 32], f32, name=f"outf{d}")
                  for d in range(2)]
        with tc.tile_pool(name="lft", bufs=2, space="PSUM") as lft:
            for d in range(2):
                pf = lft.tile([128, NT_OUT // 2, 32], fp16, tag="pf",
                              name="pf")
                for i in range(NT_OUT // 2):
                    nc.tensor.transpose(out=pf[:, i, :],
                                        in_=t_OUTH[d][:, 2 * i:2 * i + 2, :],
                                        identity=t_id128[0:32, 0:32])
                nc.vector.tensor_copy(out=t_OUTF[d][:], in_=pf[:])

        for d in range(2):
            for tp in range(2):
                nc.sync.dma_start(
                    out=bass.AP(tensor=d_out,
                                offset=tp * 64 + d * 32,
                                ap=[[NT_OUT * 64, S], [2 * 64, NT_OUT // 2],
                                    [1, 32]]),
                    in_=t_OUTF[d][tp * 64:tp * 64 + 64, :, :])

    nc.compile()
    return nc


def _get_nc(probe_layer=-1):
    key = ("nc", probe_layer)
    if key not in _cache:
        _cache[key] = _build(probe_layer)
    return _cache[key]


def run_on_cores(inputs, probe_layer=-1, trace=False):
    """Build (cached), run on 8 cores; returns (results, BassKernelResults)."""
    from concourse.bass_utils import run_bass_kernel_spmd

    nc = _get_nc(probe_layer)
    in_maps = _prep_host(inputs)
    last_exc = None
    for _ in range(3):
        try:
            res = run_bass_kernel_spmd(nc, in_maps,
                                       core_ids=list(range(N_CORES)),
                                       trace=trace)
            return res
        except Exception as e:  # transient NRT errors happen; retry
            last_exc = e
    raise last_exc


def assemble_output(res):
    out = np.zeros((B, NT_OUT, 64), np.float32)
    for core in range(N_CORES):
        s0 = core * S
        out[s0:s0 + S] = res.results[core]["out"]
    return out


def kernel(**inputs):
    res = run_on_cores(inputs)
    return assemble_output(res)



# revision 28
# speedup vs baseline: 1.1102x; 1.0108x over previous
"""Trainium2 Bass kernel for nn_Encoder_6 (conv+GN+InterpLnr x3 -> biLSTM).

Self-contained: host-side prep (sharding, interp gather tables, weight
repacking) + Bass/Tile device kernel + output gather.

Data-parallel over 8 NeuronCores: 64 samples per core.

Device dataflow per core (all samples resident on-chip after one load):
  - activations live in [channel(partition), sample, time] layout
  - conv1d = 10-11 accumulating fp16 matmuls per sample-pair (taps x
    cin-chunks), PSUM [128, 2x192]
  - GroupNorm stats from per-sample DVE bn_stats on the conv PSUM; group
    reduce/expand via tiny matmuls; normalize+ReLU = one ACT op per
    (sample, half) with per-partition scale/bias
  - InterpLnr = per-sample banded time-warp matrix matmul (PE transposes
    h, then [t_in, t_out] fp16 matmuls)
  - the six (layer, group) stages are software-pipelined: stage k+1's conv
    matmuls emit pair-interleaved with stage k's normalize/interp so the
    PE never drains (consecutive stages touch disjoint sample groups)
  - biLSTM in gate-partition layout [gate/h row, sample]: xw preacts
    batched 8 steps per PSUM bank (free-dim-512 matmuls), recurrence
    matmul consumes h [h, s] directly (no transpose); partition
    re-alignment via ACT/DVE output shifts; two independent dir chains
"""
import sys
from contextlib import ExitStack

sys.path.insert(0, "/opt/trn_rl_repo")

import numpy as np
import ml_dtypes

B = 512
N_CORES = 8
S = B // N_CORES          # samples per core
DIM_PIT = 257
C = 256                   # conv channels
T = 192                   # padded time
TH = 196                  # time with halo (2 each side)
GRP = 16                  # channels per group
DIM_NECK = 32
FREQ = 8
NT_OUT = 24               # output timesteps per direction
MIN_LEN_SEG = 19
MAX_NUM_SEG = 7
W64 = 64                  # 2*MAX_LEN_SEG
EPS = 1e-5
SG = 32                   # samples per stats group (2 groups per core)
NPAIR = 16                # sample pairs per stats group

_cache = {}


# ---------------------------------------------------------------- host prep

def _interp_tables(scales_u, len_seg_raw, n):
    """Gather idx/w1/w2 per sample for one interp layer (numpy, exact)."""
    scales = scales_u.astype(np.float32) + np.float32(0.5)
    j = np.arange(W64, dtype=np.float32)
    idx_scaled = j[None, :] / scales[:, None]
    idx_fl = np.floor(idx_scaled)
    lam = idx_scaled - idx_fl
    len_seg = (len_seg_raw + MIN_LEN_SEG).astype(np.float32)[:, None]
    idx_mask = idx_fl < (len_seg - 1.0)
    ls = (len_seg_raw + MIN_LEN_SEG).reshape(n, MAX_NUM_SEG)
    offset = np.cumsum(ls, axis=-1)
    offset = np.pad(offset[:, :-1], ((0, 0), (1, 0))).reshape(-1, 1)
    idx_org = idx_fl + offset.astype(np.float32)
    mask = (idx_mask & (idx_org < (T - 1))).reshape(n, MAX_NUM_SEG * W64)
    idx_b = np.clip(idx_org.reshape(n, -1).astype(np.int32), 0, T - 2)
    lam_b = lam.reshape(n, -1)
    idx = np.zeros((n, T), np.int32)
    w1 = np.zeros((n, T), np.float32)
    w2 = np.zeros((n, T), np.float32)
    for b in range(n):
        js = np.nonzero(mask[b])[0][:T]
        k = len(js)
        idx[b, :k] = idx_b[b, js]
        w1[b, :k] = 1.0 - lam_b[b, js]
        w2[b, :k] = lam_b[b, js]
    return idx, w1, w2


def _wrap_idx(idx_pairs):
    """[n, NI] int -> ap_gather wrapped layout [n, 128, NI//16] int16."""
    n, NI = idx_pairs.shape
    wrapped = idx_pairs.reshape(n, NI // 16, 16).transpose(0, 2, 1)
    out = np.tile(wrapped[:, None, :, :], (1, 8, 1, 1)).reshape(n, 128, NI // 16)
    return np.ascontiguousarray(out.astype(np.int16))


def _prep_host(inputs):
    """Build per-core input dicts. Returns list of 8 dicts."""
    x = np.asarray(inputs["x"], np.float32)
    scales = np.asarray(inputs["scales"], np.float32)
    lsr = np.asarray(inputs["len_seg_raw"], np.int32)

    # conv weights as lhsT tiles [l, chunk, tap, half, cin128, cout128]
    wconv = np.zeros((3, 2, 5, 2, 128, 128), np.float32)
    for l in range(3):
        w = np.asarray(inputs[f"conv{l}_w"], np.float32)  # [256, cin, 5]
        for cc in range(2):
            for k in range(5):
                for h in range(2):
                    wconv[l, cc, k, h] = w[h * 128:(h + 1) * 128,
                                           cc * 128:(cc + 1) * 128, k].T
    wconv = np.ascontiguousarray(wconv.astype(np.float16))
    # conv0 channel 256 as [5, 256] lhsT (k=tap)
    w0 = np.asarray(inputs["conv0_w"], np.float32)
    wc0e = np.ascontiguousarray(w0[:, 256, :].T.astype(np.float16))  # [5, 256]

    conv_bias = [np.asarray(inputs[f"conv{l}_b"], np.float32) for l in range(3)]
    assert all(np.abs(b).max() == 0.0 for b in conv_bias), \
        "nonzero conv bias not implemented in device kernel"

    gamma_t = np.stack([np.asarray(inputs[f"gn{l}_g"], np.float32).reshape(2, 128)
                        for l in range(3)])          # [3, 2, 128]
    beta_t = np.stack([np.asarray(inputs[f"gn{l}_b"], np.float32).reshape(2, 128)
                       for l in range(3)])
    gamma_t = np.ascontiguousarray(gamma_t.transpose(2, 0, 1).reshape(128, 6))
    beta_t = np.ascontiguousarray(beta_t.transpose(2, 0, 1).reshape(128, 6))

    gind = np.zeros((128, 8), np.float32)
    for c in range(128):
        gind[c, c // 16] = 1.0
    gexp = np.ascontiguousarray(gind.T)               # [8, 128]

    # interp tables, all samples
    idx_all, w1_all, w2_all = [], [], []
    for l in range(3):
        idx, w1, w2 = _interp_tables(scales[l], lsr[l], B)
        idx_all.append(idx)
        w1_all.append(w1)
        w2_all.append(w2)

    # LSTM weights: gate reorder i,f,o,g; gate-partition layouts
    def reord(a):  # [128, ...] gate-major
        i_, f_, g_, o_ = np.split(a, 4, axis=0)
        return np.concatenate([i_, f_, o_, g_], axis=0)

    # gate-partition LSTM layouts (state kept as [gate/h, sample]):
    #  wihg [cin128, d, cc, gate]  lhsT of xw matmuls (fp16)
    #  whhs [h, d, gate]           lhsT of recurrence matmuls (fp16)
    #  lbias [gate, d]             per-partition ACT bias (f32)
    wihg = np.zeros((128, 2, 2, 128), np.float32)
    whhs = np.zeros((32, 2, 128), np.float32)
    lbias = np.zeros((128, 2), np.float32)
    for d, nm in enumerate(["f", "b"]):
        wi = reord(np.asarray(inputs[f"w_ih_{nm}"], np.float32))   # [128, 256]
        wh = reord(np.asarray(inputs[f"w_hh_{nm}"], np.float32))   # [128, 32]
        bb = reord((np.asarray(inputs[f"b_ih_{nm}"], np.float32)
                    + np.asarray(inputs[f"b_hh_{nm}"], np.float32))[:, None])[:, 0]
        for cc in range(2):
            wihg[:, d, cc, :] = wi[:, cc * 128:(cc + 1) * 128].T
        whhs[:, d, :] = wh.T
        lbias[:, d] = bb
    wihg = np.ascontiguousarray(wihg.astype(np.float16))
    whhs = np.ascontiguousarray(whhs.astype(np.float16))
    lbias = np.ascontiguousarray(lbias)

    in_maps = []
    for core in range(N_CORES):
        s0 = core * S
        xs = x[s0:s0 + S]                              # [S, 257, 192]
        xt = xs.transpose(1, 0, 2)                     # [257, S, 192]
        xa = np.zeros((128, S, TH), np.float32)
        xb = np.zeros((128, S, TH), np.float32)
        xa[:, :, 2:194] = xt[:128]
        xb[:, :, 2:194] = xt[128:256]
        xc = np.zeros((5, S, T), np.float32)
        x256 = xt[256]                                 # [S, 192]
        for k in range(5):
            sh = k - 2
            lo, hi = max(0, -sh), min(T, T - sh)
            xc[k, :, lo:hi] = x256[:, lo + sh:hi + sh]

        # banded interp matrices S[t_in, t_out] per (layer, sample), fp16
        wS = np.zeros((3, S, T, T), np.float16)
        bi = np.arange(S)[:, None]
        pj = np.arange(T)[None, :]
        for l in range(3):
            idx = idx_all[l][s0:s0 + S]
            Sm = np.zeros((S, T, T), np.float32)
            Sm[bi, idx, pj] = w1_all[l][s0:s0 + S]
            Sm[bi, idx + 1, pj] += w2_all[l][s0:s0 + S]
            wS[l] = Sm.astype(np.float16)

        in_maps.append({
            "xa": np.ascontiguousarray(xa.astype(np.float16)),
            "xb": np.ascontiguousarray(xb.astype(np.float16)),
            "xc": np.ascontiguousarray(xc.astype(np.float16)),
            "wconv": wconv,
            "wc0e": wc0e,
            "gamma_t": gamma_t,
            "beta_t": beta_t,
            "gind": gind,
            "gexp": gexp,
            "wS": np.ascontiguousarray(wS),
            "id128": np.eye(128, dtype=np.float16),
            "si2": np.ascontiguousarray(
                np.concatenate([np.eye(32), np.eye(32)]).astype(np.float32)),
            "wihg": wihg,
            "whhs": whhs,
            "lbias": lbias,
        })
    return in_maps


# ------------------------------------------------------------- device build

def _build(probe_layer=-1):
    """Build the Bacc module. probe_layer >= 0 adds a probe output of XBUF
    after that layer's interp (for debugging)."""
    import concourse.bass as bass
    import concourse.tile as tile
    from concourse import bacc, mybir

    f32 = mybir.dt.float32
    f32r = mybir.dt.float32r
    bf16 = mybir.dt.bfloat16
    fp16 = mybir.dt.float16
    i16 = mybir.dt.int16
    AF = mybir.ActivationFunctionType
    OP = mybir.AluOpType

    nc = bacc.Bacc("TRN2", target_bir_lowering=False, debug=False,
                   enable_asserts=False, num_devices=N_CORES)

    # DRAM tensors
    d_xa = nc.dram_tensor("xa", [128, S, TH], fp16, kind="ExternalInput")
    d_xb = nc.dram_tensor("xb", [128, S, TH], fp16, kind="ExternalInput")
    d_xc = nc.dram_tensor("xc", [5, S, T], fp16, kind="ExternalInput")
    d_wconv = nc.dram_tensor("wconv", [3, 2, 5, 2, 128, 128], fp16,
                             kind="ExternalInput")
    d_wc0e = nc.dram_tensor("wc0e", [5, 256], fp16, kind="ExternalInput")
    d_gamma = nc.dram_tensor("gamma_t", [128, 6], f32, kind="ExternalInput")
    d_beta = nc.dram_tensor("beta_t", [128, 6], f32, kind="ExternalInput")
    d_gind = nc.dram_tensor("gind", [128, 8], f32, kind="ExternalInput")
    d_gexp = nc.dram_tensor("gexp", [8, 128], f32, kind="ExternalInput")
    d_wS = nc.dram_tensor("wS", [3, S, T, T], fp16, kind="ExternalInput")
    d_id128 = nc.dram_tensor("id128", [128, 128], fp16, kind="ExternalInput")
    d_si2 = nc.dram_tensor("si2", [64, 32], f32, kind="ExternalInput")
    d_wihg = nc.dram_tensor("wihg", [128, 2, 2, 128], fp16, kind="ExternalInput")
    d_whhs = nc.dram_tensor("whhs", [32, 2, 128], fp16, kind="ExternalInput")
    d_lbias = nc.dram_tensor("lbias", [128, 2], f32, kind="ExternalInput")
    d_out = nc.dram_tensor("out", [S, NT_OUT, 64], f32, kind="ExternalOutput")
    d_probe = None
    if probe_layer >= 0:
        d_probe = nc.dram_tensor("probe", [2, 128, S, TH], f32r,
                                 kind="ExternalOutput")

    es = ExitStack()
    with tile.TileContext(nc) as tc, es:
        consts = es.enter_context(tc.tile_pool(name="consts", bufs=1))
        xbufs = es.enter_context(tc.tile_pool(name="xbufs", bufs=1))

        # ---- constants
        t_xc = consts.tile([5, S, T], fp16)
        nc.sync.dma_start(out=t_xc[:], in_=d_xc[:, :, :])
        t_wc0e = consts.tile([5, 256], fp16)
        nc.sync.dma_start(out=t_wc0e[:], in_=d_wc0e[:, :])
        t_gamma = consts.tile([128, 6], f32)
        nc.sync.dma_start(out=t_gamma[:], in_=d_gamma[:, :])
        t_beta = consts.tile([128, 6], f32)
        nc.sync.dma_start(out=t_beta[:], in_=d_beta[:, :])
        t_gind = consts.tile([128, 8], f32)
        nc.sync.dma_start(out=t_gind[:], in_=d_gind[:, :])
        t_gexp = consts.tile([8, 128], f32)
        nc.sync.dma_start(out=t_gexp[:], in_=d_gexp[:, :])
        t_eps = consts.tile([8, 1], f32)
        nc.vector.memset(t_eps[:], EPS)
        t_wihg = consts.tile([128, 2, 2, 128], fp16)
        nc.sync.dma_start(out=t_wihg[:], in_=d_wihg[:, :, :, :])
        t_whhs = consts.tile([32, 2, 128], fp16)
        nc.sync.dma_start(out=t_whhs[:], in_=d_whhs[:, :, :])
        t_lbias = consts.tile([128, 2], f32)
        nc.sync.dma_start(out=t_lbias[:], in_=d_lbias[:, :])
        t_id128 = consts.tile([128, 128], fp16)
        nc.sync.dma_start(out=t_id128[:], in_=d_id128[:, :])
        t_si2 = consts.tile([64, 32], f32)
        nc.sync.dma_start(out=t_si2[:], in_=d_si2[:, :])

        # ---- input activations (xbuf reused as interp output every layer)
        t_xa = xbufs.tile([128, S, TH], fp16)
        t_xb = xbufs.tile([128, S, TH], fp16)
        nc.sync.dma_start(out=t_xa[:], in_=d_xa[:, :, :])
        nc.sync.dma_start(out=t_xb[:], in_=d_xb[:, :, :])
        xbuf = [t_xa, t_xb]

        def mm(out, lhsT, rhs, start, stop, dt=None, **kw):
            if dt is not None:
                lhsT = lhsT.bitcast(dt)
                rhs = rhs.bitcast(dt)
            nc.tensor.matmul(out=out, lhsT=lhsT, rhs=rhs, start=start,
                             stop=stop, **kw)

        # ================= conv + GN + interp layers =================
        with ExitStack() as ces:
            wpool = ces.enter_context(tc.tile_pool(name="wpool", bufs=2))
            hraw_p = ces.enter_context(tc.tile_pool(name="hraw", bufs=2))
            stats_p = ces.enter_context(tc.tile_pool(name="stats", bufs=2))
            small_p = ces.enter_context(tc.tile_pool(name="small", bufs=2))
            y_p = ces.enter_context(tc.tile_pool(name="ybuf", bufs=3))
            sm_p = ces.enter_context(tc.tile_pool(name="smat", bufs=2))
            yt_p = ces.enter_context(tc.tile_pool(name="ytp", bufs=3))
            cpsum = ces.enter_context(
                tc.tile_pool(name="cpsum", bufs=2, space="PSUM"))
            stps = ces.enter_context(
                tc.tile_pool(name="stps", bufs=2, space="PSUM"))
            tpsum = ces.enter_context(
                tc.tile_pool(name="tpsum", bufs=2, space="PSUM"))
            sops = ces.enter_context(
                tc.tile_pool(name="sops", bufs=2, space="PSUM"))

            def emit_ph1_pair(l, grp, pp, t_wc, hraw, st):
                pr = grp * NPAIR + pp
                for h in range(2):
                    ps = cpsum.tile([128, 2, T], f32, tag="cps")
                    ops = []
                    for cc in range(2):
                        for k in range(5):
                            ops.append((
                                t_wc[:, (cc * 5 + k) * 2 + h, :],
                                xbuf[cc][:, 2 * pr:2 * pr + 2, k:k + T]))
                    if l == 0:
                        ops.append((t_wc0e[:, h * 128:(h + 1) * 128],
                                    t_xc[:, 2 * pr:2 * pr + 2, :]))
                    for j, (lh, rh) in enumerate(ops):
                        mm(ps[:], lh, rh, j == 0, j == len(ops) - 1)
                    nc.scalar.activation(
                        out=hraw[h][:, 2 * pp:2 * pp + 2, :],
                        in_=ps[:, :, :], func=AF.Identity)
                    for i in range(2):
                        nc.vector.bn_stats(out=st[h][:, 2 * pp + i, :, :],
                                           in_=hraw[h][:, 2 * pp + i, :])

            def emit_ph2(l, st):
                AB = []
                for h in range(2):
                    sq = stats_p.tile([128, SG, 2], f32, tag=f"sq{h}",
                                      name=f"sq{h}")
                    nc.vector.tensor_tensor(out=sq[:], in0=st[h][:, :, :, 1],
                                            in1=st[h][:, :, :, 1], op=OP.mult)
                    g6 = stps.tile([8, SG, 2, 3], f32, tag="gg")
                    mm(g6[:], t_gind[:], st[h][:], True, True)
                    gq = stps.tile([8, SG, 2], f32, tag="gg")
                    mm(gq[:], t_gind[:], sq[:], True, True)
                    g6s = small_p.tile([8, SG, 2, 3], f32, tag="g6s")
                    nc.vector.tensor_copy(out=g6s[:], in_=g6[:])
                    gqs = small_p.tile([8, SG, 2], f32, tag="gqs")
                    nc.vector.tensor_copy(out=gqs[:], in_=gq[:])
                    s_m = small_p.tile([8, SG], f32, tag="s_m")
                    nc.vector.tensor_tensor(out=s_m[:], in0=g6s[:, :, 0, 1],
                                            in1=g6s[:, :, 1, 1], op=OP.add)
                    s_cv = small_p.tile([8, SG], f32, tag="s_cv")
                    nc.vector.tensor_tensor(out=s_cv[:], in0=g6s[:, :, 0, 2],
                                            in1=g6s[:, :, 1, 2], op=OP.add)
                    s_q = small_p.tile([8, SG], f32, tag="s_q")
                    nc.vector.tensor_tensor(out=s_q[:], in0=gqs[:, :, 0],
                                            in1=gqs[:, :, 1], op=OP.add)
                    mean = small_p.tile([8, SG], f32, tag="mean")
                    nc.vector.tensor_scalar_mul(mean[:], s_m[:],
                                                1.0 / (2 * GRP))
                    msq = small_p.tile([8, SG], f32, tag="msq")
                    nc.vector.tensor_tensor(out=msq[:], in0=mean[:],
                                            in1=mean[:], op=OP.mult)
                    t1 = small_p.tile([8, SG], f32, tag="t1")
                    nc.vector.scalar_tensor_tensor(
                        out=t1[:], in0=s_q[:], scalar=1.0 / (2 * GRP),
                        in1=msq[:], op0=OP.mult, op1=OP.subtract)
                    var = small_p.tile([8, SG], f32, tag="var")
                    nc.vector.scalar_tensor_tensor(
                        out=var[:], in0=s_cv[:], scalar=1.0 / (GRP * T),
                        in1=t1[:], op0=OP.mult, op1=OP.add)
                    sd = small_p.tile([8, SG], f32, tag="sd")
                    nc.scalar.activation(out=sd[:], in_=var[:], func=AF.Sqrt,
                                         bias=t_eps[:, :1], scale=1.0)
                    rstd = small_p.tile([8, SG], f32, tag="rstd")
                    nc.vector.reciprocal(rstd[:], sd[:])
                    rp = stps.tile([128, SG], f32, tag="gg")
                    mm(rp[:], t_gexp[:], rstd[:], True, True)
                    mp = stps.tile([128, SG], f32, tag="gg")
                    mm(mp[:], t_gexp[:], mean[:], True, True)
                    At = small_p.tile([128, SG], f32, tag="A")
                    nc.vector.tensor_scalar_mul(
                        At[:], rp[:], t_gamma[:, l * 2 + h:l * 2 + h + 1])
                    tmp = small_p.tile([128, SG], f32, tag="tmp")
                    nc.vector.tensor_tensor(out=tmp[:], in0=mp[:], in1=At[:],
                                            op=OP.mult)
                    Bt = small_p.tile([128, SG], f32, tag="B")
                    nc.vector.tensor_scalar(
                        out=Bt[:], in0=tmp[:], scalar1=-1.0,
                        scalar2=t_beta[:, l * 2 + h:l * 2 + h + 1],
                        op0=OP.mult, op1=OP.add)
                    AB.append((At, Bt))
                return AB

            def emit_ph3_pair(l, grp, pp, hraw, AB):
                pr = grp * NPAIR + pp
                s128 = sm_p.tile([128, 2, T], fp16, tag="s128")
                nc.gpsimd.dma_start(
                    out=s128[:],
                    in_=d_wS[l, 2 * pr:2 * pr + 2, 0:128, :].rearrange(
                        "s t w -> t s w"))
                s64 = sm_p.tile([64, 2, T], fp16, tag="s64")
                nc.gpsimd.dma_start(
                    out=s64[:],
                    in_=d_wS[l, 2 * pr:2 * pr + 2, 128:192, :].rearrange(
                        "s t w -> t s w"))
                for i in range(2):
                    sl = pp * 2 + i
                    sg_ = 2 * pr + i
                    yt128 = yt_p.tile([128, 2, 128], fp16, tag="yt128")
                    yt64 = yt_p.tile([64, 2, 128], fp16, tag="yt64")
                    ptp = tpsum.tile([128, 2, 2, 128], fp16, tag="tp",
                                     name="ptp")
                    for h in range(2):
                        At, Bt = AB[h]
                        yb = y_p.tile([128, T], fp16, tag="yb")
                        nc.scalar.activation(
                            out=yb[:], in_=hraw[h][:, sl, :], func=AF.Relu,
                            bias=Bt[:, sl:sl + 1], scale=At[:, sl:sl + 1])
                        nc.tensor.transpose(out=ptp[:, h, 0, :],
                                            in_=yb[:, 0:128],
                                            identity=t_id128[:])
                        nc.tensor.transpose(out=ptp[0:64, h, 1, :],
                                            in_=yb[:, 128:192],
                                            identity=t_id128[:])
                    nc.scalar.activation(out=yt128[:], in_=ptp[:, :, 0, :],
                                         func=AF.Identity)
                    nc.vector.tensor_copy(out=yt64[:], in_=ptp[0:64, :, 1, :])
                    sout = sops.tile([128, 2, T], f32, tag="so", name="sout")
                    for ch in range(2):
                        mm(sout[:, ch, :], yt128[:, ch, :], s128[:, i, :],
                           True, False)
                        mm(sout[:, ch, :], yt64[:, ch, :], s64[:, i, :],
                           False, True)
                    nc.vector.tensor_copy(out=xbuf[0][:, sg_, 2:194],
                                          in_=sout[:, 0, :])
                    nc.vector.tensor_copy(out=xbuf[1][:, sg_, 2:194],
                                          in_=sout[:, 1, :])

            def emit_probe(l):
                if probe_layer == l:
                    for h in range(2):
                        nc.sync.dma_start(out=d_probe[h, :, :, :],
                                          in_=xbuf[h][:, :, :])

            # software pipeline over the 6 (layer, group) stages: the conv
            # matmuls of stage k+1 are emitted pair-interleaved with the
            # normalize/interp work of stage k so the PE never drains
            # (consecutive stages touch disjoint sample groups).
            prev = None
            t_wc = None
            for l in range(3):
                for grp in range(2):
                    if grp == 0:
                        t_wc = wpool.tile([128, 20, 128], fp16, tag="wconv")
                        nc.sync.dma_start(
                            out=t_wc[:],
                            in_=bass.AP(tensor=d_wconv,
                                        offset=l * 20 * 128 * 128,
                                        ap=[[128, 128], [128 * 128, 20],
                                            [1, 128]]))
                    st = [stats_p.tile([128, SG, 2, 3], f32, tag=f"st{h}",
                                       name=f"st{h}") for h in range(2)]
                    hraw = [hraw_p.tile([128, SG, T], fp16, tag=f"hraw{h}",
                                        name=f"hraw{h}") for h in range(2)]
                    for pp in range(NPAIR):
                        if prev is not None:
                            emit_ph3_pair(prev[0], prev[1], pp, prev[2],
                                          prev[3])
                        emit_ph1_pair(l, grp, pp, t_wc, hraw, st)
                    if prev is not None and prev[1] == 1:
                        emit_probe(prev[0])
                    AB = emit_ph2(l, st)
                    prev = (l, grp, hraw, AB)
            for pp in range(NPAIR):
                emit_ph3_pair(prev[0], prev[1], pp, prev[2], prev[3])
            emit_probe(2)

        # ======================= biLSTM =======================
        # gate-partition layout: gates/state live as [gate-or-h row, sample].
        # Gate rows in PSUM preact banks: i 0:32 | f 32:64 | o 64:96 | g 96:128.
        # Partition re-alignment is done by ACT/DVE output shifts (legal: only
        # DVE *inputs* must share a start partition); P_i/P_f are shifted onto
        # the same rows so c' = P_i + P_f is a plain DVE add.
        # xw preacts batched TB timesteps per PSUM bank (matmul free dim 512).
        TB = 8
        NB8 = T // TB            # 24 banks per direction
        lsb = es.enter_context(tc.tile_pool(name="lstm_sbuf", bufs=1))
        t_SIG = [lsb.tile([96, S], fp16, name=f"sig{d}") for d in range(2)]
        t_TG = [lsb.tile([32, S], fp16, name=f"tg{d}") for d in range(2)]
        # P2: [pi/pf, s] both at rows 0:32 (DVE output-shift realigns P_f)
        t_P2 = [lsb.tile([32, 2, S], fp16, name=f"pp{d}") for d in range(2)]
        # CC2 rows 32:64 = c for both dirs (aligned with sigma_f rows);
        # shared so tanh(c) is one ACT op covering both chains
        t_CC2 = lsb.tile([64, 2, S], fp16, name="cc2")
        t_TC2 = lsb.tile([96, 2, S], fp16, name="tc2")
        t_H = [lsb.tile([32, S], fp16, name=f"hh{d}") for d in range(2)]
        t_OUTH = [lsb.tile([32, NT_OUT, S], fp16, name=f"outh{d}")
                  for d in range(2)]
        for d in range(2):
            nc.vector.memset(t_H[d][:], 0.0)
        nc.vector.memset(t_CC2[32:64, :, :], 0.0)

        with ExitStack() as les:
            lxw = les.enter_context(
                tc.tile_pool(name="lxw", bufs=2, space="PSUM"))

            def xw_block(b, d):
                """gate preacts for timesteps 8b..8b+7 (natural order)."""
                ps = lxw.tile([128, S, TB], f32, tag=f"xw{d}", name=f"xw{d}")
                for cc in range(2):
                    mm(ps[:], t_wihg[:, d, cc, :],
                       xbuf[cc][:, :, 2 + TB * b:2 + TB * b + TB],
                       cc == 0, False)
                return ps

            cur = [xw_block(0, 0), xw_block(NB8 - 1, 1)]
            nxt = [None, None]

            for g in range(T):
                tt = [g, T - 1 - g]
                bj = [(t // TB, t % TB) for t in tt]
                # bank swap on entry (fwd enters at j==0, bwd at j==7)
                for d in range(2):
                    if g > 0 and bj[d][1] == (0 if d == 0 else TB - 1):
                        cur[d] = nxt[d]
                # two fully-independent dir chains, emitted in lockstep
                for d in range(2):
                    b, j = bj[d]
                    mm(cur[d][:, :, j], t_whhs[:, d, :], t_H[d][:], False,
                       j == (TB - 1 if d == 0 else 0), skip_group_check=True)
                for d in range(2):
                    j = bj[d][1]
                    nc.scalar.activation(out=t_SIG[d][:], in_=cur[d][0:96, :, j],
                                         func=AF.Sigmoid,
                                         bias=t_lbias[0:96, d:d + 1])
                for d in range(2):
                    j = bj[d][1]
                    nc.scalar.activation(out=t_TG[d][:],
                                         in_=cur[d][96:128, :, j],
                                         func=AF.Tanh,
                                         bias=t_lbias[96:128, d:d + 1])
                for d in range(2):
                    nc.vector.tensor_tensor(out=t_P2[d][:, 0, :],
                                            in0=t_SIG[d][0:32, :],
                                            in1=t_TG[d][:], op=OP.mult)
                for d in range(2):
                    nc.vector.tensor_tensor(out=t_P2[d][:, 1, :],
                                            in0=t_SIG[d][32:64, :],
                                            in1=t_CC2[32:64, d, :],
                                            op=OP.mult)
                for d in range(2):
                    nc.vector.tensor_tensor(out=t_CC2[32:64, d, :],
                                            in0=t_P2[d][:, 0, :],
                                            in1=t_P2[d][:, 1, :], op=OP.add)
                nc.scalar.activation(out=t_TC2[64:96, :, :],
                                     in_=t_CC2[32:64, :, :], func=AF.Tanh)
                for d in range(2):
                    nc.vector.tensor_tensor(out=t_H[d][:],
                                            in0=t_SIG[d][64:96, :],
                                            in1=t_TC2[64:96, d, :],
                                            op=OP.mult)
                for d in range(2):
                    t = tt[d]
                    if (d == 0 and t % FREQ == FREQ - 1) or \
                       (d == 1 and t % FREQ == 0):
                        nc.vector.tensor_copy(out=t_OUTH[d][:, t // FREQ, :],
                                              in_=t_H[d][:])
                # prefetch next banks early in each bank's life
                if bj[0][1] == 1 and bj[0][0] + 1 < NB8:
                    nxt[0] = xw_block(bj[0][0] + 1, 0)
                if bj[1][1] == TB - 2 and bj[1][0] - 1 >= 0:
                    nxt[1] = xw_block(bj[1][0] - 1, 1)

        # ---- emit outputs: transpose [32 h, (t-pair, s)] -> [(tp, s), h]
        t_OUTF = [lsb.tile([128, NT_OUT // 2, 32], f32, name=f"outf{d}")
                  for d in range(2)]
        with tc.tile_pool(name="lft", bufs=2, space="PSUM") as lft:
            for d in range(2):
                pf = lft.tile([128, NT_OUT // 2, 32], fp16, tag="pf",
                              name="pf")
                for i in range(NT_OUT // 2):
                    nc.tensor.transpose(out=pf[:, i, :],
                                        in_=t_OUTH[d][:, 2 * i:2 * i + 2, :],
                                        identity=t_id128[0:32, 0:32])
                nc.vector.tensor_copy(out=t_OUTF[d][:], in_=pf[:])

        for d in range(2):
            for tp in range(2):
                nc.sync.dma_start(
                    out=bass.AP(tensor=d_out,
                                offset=tp * 64 + d * 32,
                                ap=[[NT_OUT * 64, S], [2 * 64, NT_OUT // 2],
                                    [1, 32]]),
                    in_=t_OUTF[d][tp * 64:tp * 64 + 64, :, :])

    nc.compile()
    return nc


def _get_nc(probe_layer=-1):
    key = ("nc", probe_layer)
    if key not in _cache:
        _cache[key] = _build(probe_layer)
    return _cache[key]


def run_on_cores(inputs, probe_layer=-1, trace=False):
    """Build (cached), run on 8 cores; returns (results, BassKernelResults)."""
    from concourse.bass_utils import run_bass_kernel_spmd

    nc = _get_nc(probe_layer)
    in_maps = _prep_host(inputs)
    last_exc = None
    for _ in range(3):
        try:
            res = run_bass_kernel_spmd(nc, in_maps,
                                       core_ids=list(range(N_CORES)),
                                       trace=trace)
            return res
        except Exception as e:  # transient NRT errors happen; retry
            last_exc = e
    raise last_exc


def assemble_output(res):
    out = np.zeros((B, NT_OUT, 64), np.float32)
    for core in range(N_CORES):
        s0 = core * S
        out[s0:s0 + S] = res.results[core]["out"]
    return out


def kernel(**inputs):
    res = run_on_cores(inputs)
    return assemble_output(res)

